# revision 1
# baseline (speedup 1.0000x reference)
"""Trainium2 Bass kernel v2 for DeepHedgingModel (LSTM scan, B=8192 T=512 F=4 H=32).

Pure data parallel over 8 cores, 1024 batch rows per core.

Layout (per core): 4 "bands" of 256 batch columns; band j owns SBUF/PSUM
partitions [32j, 32j+32) for the h/c state and gate tensors.  The z vector
(d, x0..x3, ones) for band j lives on partitions [6j, 6j+6) of a compact
24-partition tile, so every row is initialized (no NaN x 0 poison).

All per-gate matmuls cover all 4 bands in ONE instruction via
block-diagonal stationary weights:
  G_g [128, W] = wx_g[24,128].T @ z[24, W]  (start)
               + whh_g[128,128].T @ h[128, W]  (stop)
with wx_g / whh_g zero outside the band-diagonal blocks.  Matmul operands
are bf16 (1 cycle/row on PE); PSUM accumulates fp32; the c state stays
fp32.

Gate-type column order in G/S is (g, i, f, o): one Tanh covers cols 0:W,
one Sigmoid covers W:4W.

The batch columns are split into 2 phases of W=128 so two independent
dependency chains interleave on the engines.  h = sig(o)*tanh(c) runs on
the Pool engine to offload DVE.

d-feedback: sigma(y) = 0.5 + 0.5*tanh(y/2); the raw tanh value is stored
as the recurrent "d" row (bf16) and the affine is folded into the d-row
weights + bias; the (Y + b2)/2 scale/bias ride the final Activation.
Output: d rows of the z tiles are DMA-dumped per x-block; host transposes
and applies 0.5 + 0.5*x.
"""

import sys
from contextlib import ExitStack

import numpy as np

sys.path.insert(0, "/opt/trn_rl_repo")

import ml_dtypes  # noqa: E402

import concourse.tile as tile  # noqa: E402
from concourse import bacc, mybir  # noqa: E402

F32 = mybir.dt.float32
BF16 = mybir.dt.bfloat16
AF = mybir.ActivationFunctionType
ALU = mybir.AluOpType

EPS = 1e-5
BF = ml_dtypes.bfloat16


# ----------------------------------------------------------------------------
# Config
# ----------------------------------------------------------------------------
class Cfg:
    def __init__(self, ncol=256, T=512, sblk=16, nph=2):
        self.ncol = ncol          # batch columns per band
        self.T = T                # timesteps
        self.sblk = sblk          # steps per x-DMA block
        self.nph = nph            # column phases per step
        self.nbands = 4
        self.B = 4 * ncol         # per-core batch
        assert T % sblk == 0 and ncol % nph == 0


FULL = Cfg()


# ----------------------------------------------------------------------------
# Host-side weight folding / input prep
# ----------------------------------------------------------------------------
def fold_params(p):
    """Block-diagonal bf16 weights for the SBUF-resident constants."""
    H = 32
    W_ih = p["W_ih"].astype(np.float64)
    W_hh = p["W_hh"].astype(np.float64)
    b_ih = p["b_ih"].astype(np.float64)
    b_hh = p["b_hh"].astype(np.float64)
    gam = p["bn_gamma"].astype(np.float64)
    bet = p["bn_beta"].astype(np.float64)
    mu = p["bn_mean"].astype(np.float64)
    var = p["bn_var"].astype(np.float64)
    a = gam / np.sqrt(var + EPS)          # [5]
    b_a = bet - mu * a                    # [5]

    Wx_eff = W_ih[:, :4] * a[None, :4]    # [128, 4]
    w_d_eff = W_ih[:, 4] * a[4]           # [128]
    C = b_a @ W_ih.T + b_ih + b_hh + 0.5 * w_d_eff   # [128]
    d_row = 0.5 * w_d_eff                 # weight on stored t_y = tanh(y/2)

    # col order of gate blocks: (g, f, i, o); torch rows are (i, f, g, o)
    blocks = [slice(2 * H, 3 * H), slice(H, 2 * H), slice(0, H), slice(3 * H, 4 * H)]

    out = {"wx": [], "whh": [], "wxd": []}
    for g in range(4):
        rows = blocks[g]
        # wx_g [20, 128]: band j rows 5j..5j+4 (x0..x3, ones) -> cols 32j..
        wx = np.zeros((20, 128))
        # wxd_g [4, 128]: band j row j = d-row weight -> cols 32j..
        wxd = np.zeros((4, 128))
        # whh_g [128, 128]: band j rows 32j+k -> cols 32j+m
        wh = np.zeros((128, 128))
        for j in range(4):
            wx[5 * j : 5 * j + 4, 32 * j : 32 * j + 32] = Wx_eff[rows, :].T
            wx[5 * j + 4, 32 * j : 32 * j + 32] = C[rows]
            wxd[j, 32 * j : 32 * j + 32] = d_row[rows]
            wh[32 * j : 32 * j + 32, 32 * j : 32 * j + 32] = W_hh[rows, :].T
        out["wx"].append(wx.astype(BF))
        out["wxd"].append(wxd.astype(BF))
        out["whh"].append(wh.astype(BF))

    W1 = p["W1"].astype(np.float64)       # [32, 32] (D1, H)
    wm1 = np.zeros((128, 128))
    wm2 = np.zeros((128, 4))
    W2 = p["W2"].astype(np.float64)       # [1, 32]
    for j in range(4):
        wm1[32 * j : 32 * j + 32, 32 * j : 32 * j + 32] = W1.T
        wm2[32 * j : 32 * j + 32, j] = W2[0]
    out["wm1"] = wm1.astype(BF)
    out["wm2"] = wm2.astype(BF)
    out["b1col"] = np.tile(p["b1"].astype(np.float32)[:, None], (4, 1))  # [128,1]
    out["b2half"] = float(0.5 * p["b2"].astype(np.float64)[0])
    return out


def prep_x(x_core, cfg):
    """x_core [B, T, 4] -> xprep [T/sblk, 4, 5, sblk, ncol] bf16 (ones row 4)."""
    B, T, F = x_core.shape
    nc_, sb = cfg.ncol, cfg.sblk
    xp = np.empty((T // sb, 4, 5, sb, nc_), BF)
    xr = x_core.reshape(4, nc_, T // sb, sb, F)
    xp[:, :, :4] = xr.transpose(2, 0, 4, 3, 1).astype(BF)  # [blk, band, f, s, n]
    xp[:, :, 4] = BF(1.0)
    return xp


# ----------------------------------------------------------------------------
# Kernel body
# ----------------------------------------------------------------------------
def build_kernel(nc, cfg, time_mode=False):
    """Declare DRAM I/O and emit the TileContext program.

    time_mode=True shrinks xprep to one block that every step re-reads —
    wrong math, identical instruction stream — to measure device time
    without the axon per-call input-transfer cost.
    """
    N = cfg.ncol
    T, SB = cfg.T, cfg.sblk
    NPH = cfg.nph
    W = N // NPH

    nxblk = 1 if time_mode else T // SB
    d_x = nc.dram_tensor("xprep", [nxblk, 4, 5, SB, N], BF16, kind="ExternalInput")
    d_wx = [nc.dram_tensor(f"wx{g}", [20, 128], BF16, kind="ExternalInput")
            for g in range(4)]
    d_wxd = [nc.dram_tensor(f"wxd{g}", [4, 128], BF16, kind="ExternalInput")
             for g in range(4)]
    d_whh = [nc.dram_tensor(f"whh{g}", [128, 128], BF16, kind="ExternalInput")
             for g in range(4)]
    d_wm1 = nc.dram_tensor("wm1", [128, 128], BF16, kind="ExternalInput")
    d_wm2 = nc.dram_tensor("wm2", [128, 4], BF16, kind="ExternalInput")
    d_b1 = nc.dram_tensor("b1col", [128, 1], F32, kind="ExternalInput")
    d_b2h = nc.dram_tensor("b2half", [128, 1], F32, kind="ExternalInput")
    # raw tanh(y/2) history (bf16), dumped per x-block; host transposes+affine
    d_out = nc.dram_tensor("d_scratch", [T // SB + 1, 4, SB * N], BF16,
                           kind="ExternalOutput")

    with tile.TileContext(nc) as tc, ExitStack() as ctx:
        wp = ctx.enter_context(tc.tile_pool(name="weights", bufs=1))
        zp = ctx.enter_context(tc.tile_pool(name="zb", bufs=3))
        sp = ctx.enter_context(tc.tile_pool(name="sgate", bufs=2))
        cp = ctx.enter_context(tc.tile_pool(name="state", bufs=1))
        tp = ctx.enter_context(tc.tile_pool(name="tmp", bufs=2))
        pg = ctx.enter_context(tc.tile_pool(name="psum_g", bufs=2, space="PSUM"))
        pm = ctx.enter_context(tc.tile_pool(name="psum_m", bufs=1, space="PSUM"))
        py = ctx.enter_context(tc.tile_pool(name="psum_y", bufs=1, space="PSUM"))

        # --- constants into SBUF
        wx = []
        wxd = []
        whh = []
        for g in range(4):
            t = wp.tile([20, 128], BF16, tag=f"wx{g}")
            nc.sync.dma_start(t[:], d_wx[g][:])
            wx.append(t)
            t = wp.tile([36, 128], BF16, tag=f"wxd{g}")
            nc.sync.dma_start(t[32:36, :], d_wxd[g][:])
            wxd.append(t)
            t = wp.tile([128, 128], BF16, tag=f"whh{g}")
            nc.sync.dma_start(t[:], d_whh[g][:])
            whh.append(t)
        wm1 = wp.tile([128, 128], BF16)
        nc.sync.dma_start(wm1[:], d_wm1[:])
        wm2 = wp.tile([128, 4], BF16)
        nc.sync.dma_start(wm2[:], d_wm2[:])
        b1c = wp.tile([128, 1], F32)
        nc.sync.dma_start(b1c[:], d_b1[:])
        b2h = wp.tile([128, 1], F32)
        nc.sync.dma_start(b2h[:], d_b2h[:])

        # --- state
        cst = cp.tile([128, N], F32)
        nc.vector.memset(cst[:], 0.0)
        hst = cp.tile([128, N], BF16)
        nc.vector.memset(hst[:], 0.0)
        dtail = cp.tile([36, N], BF16)

        # --- x block tiles, created on demand (prefetched one block early)
        zb_tiles = {}

        def get_zb(blk):
            if blk not in zb_tiles:
                zt = zp.tile([36, SB * N], BF16, tag="zb")
                xblk = 0 if time_mode else blk
                for j in range(4):
                    nc.sync.dma_start(zt[5 * j : 5 * j + 5, :], d_x[xblk, j])
                zb_tiles[blk] = zt
                if len(zb_tiles) > 3:
                    del zb_tiles[min(zb_tiles)]
            return zb_tiles[blk]

        def drows(zt):
            """AP of the 4 d rows (partitions 32..35)."""
            return zt[32:36]

        z0 = get_zb(0)
        # d(t=0) stored value: tanh form of d=0 is -1.  Phase A's half is a
        # plain memset; phase B's half artificially depends on phase A's
        # first gate activations so the two chains start ~half a step
        # apart and stay offset (they are otherwise independent).
        nc.vector.memset(drows(z0)[:, 0 : N // NPH], -1.0)
        stagger = [drows(z0)[:, N // NPH : N]]

        def emit_step(t, PH):
            """Emit one timestep for the given phase list.

            Emission order = engine FIFO tie-break; instructions are
            emitted in dependency waves so neither phase's ready work
            queues behind the other phase's not-yet-ready work.
            """
            blk, s = divmod(t, SB)
            zbt = get_zb(blk)
            if s == 0 and blk + 1 < T // SB:
                get_zb(blk + 1)  # prefetch next x block
            if t + 1 < T:
                nblk, ns = divmod(t + 1, SB)
                zbn, nscol = get_zb(nblk), ns
            else:
                zbn, nscol = dtail, 0

            col = {ph: slice(ph * W, (ph + 1) * W) for ph in PH}      # state
            zcol = {ph: slice(s * N + ph * W, s * N + (ph + 1) * W)
                    for ph in PH}
            ncolr = {ph: slice(nscol * N + ph * W, nscol * N + (ph + 1) * W)
                     for ph in PH}

            # ---- gates: G[128, 4W] cols (g,f,i,o), all 4 bands at once
            M1 = {ph: pm.tile([128, W], F32, tag=f"M1{ph}", name=f"M1{ph}")
                  for ph in PH}
            G = {ph: pg.tile([128, 4 * W], F32, tag=f"G{ph}", name=f"G{ph}")
                 for ph in PH}
            # One accumulation group per G tile: the first matmul's
            # start=True marks the tile's whole 2KB zero region pending;
            # every later matmul first-touch-replaces, so accumulation
            # order across column blocks doesn't matter.  stop rides the
            # last d-matmul.
            for ph in PH:
                for g in range(4):
                    gc = slice(g * W, (g + 1) * W)
                    nc.tensor.matmul(G[ph][:, gc], wx[g][:],
                                     zbt[0:20, zcol[ph]],
                                     start=(g == 0), stop=False)
                    nc.tensor.matmul(G[ph][:, gc], whh[g][:], hst[:, col[ph]],
                                     start=False, stop=False)
            # d contribution last: frees the bulk matmuls from waiting on d
            for ph in PH:
                for g in range(4):
                    gc = slice(g * W, (g + 1) * W)
                    nc.tensor.matmul(G[ph][:, gc], wxd[g][32:36, :],
                                     zbt[32:36, zcol[ph]],
                                     start=False, stop=(g == 3))

            # ---- activations on gates (sig(f,i) first: feeds the c chain)
            S = {ph: sp.tile([128, 4 * W], BF16, tag=f"S{ph}", name=f"S{ph}")
                 for ph in PH}
            for ph in PH:
                nc.scalar.activation(S[ph][:, W : 4 * W], G[ph][:, W : 4 * W],
                                     AF.Sigmoid)
                nc.scalar.activation(S[ph][:, 0:W], G[ph][:, 0:W], AF.Tanh)

            # ---- c update (cols: 0:W=tg, W:2W=sf, 2W:3W=si, 3W:4W=so)
            q = {ph: tp.tile([128, W], F32, tag=f"q{ph}", name=f"q{ph}")
                 for ph in PH}
            pp = {ph: tp.tile([128, W], BF16, tag=f"p{ph}", name=f"p{ph}")
                  for ph in PH}
            for ph in PH:
                nc.vector.tensor_mul(q[ph][:], S[ph][:, W : 2 * W],
                                     cst[:, col[ph]])
                nc.vector.tensor_mul(pp[ph][:], S[ph][:, 2 * W : 3 * W],
                                     S[ph][:, 0:W])
                nc.vector.tensor_add(cst[:, col[ph]], q[ph][:], pp[ph][:])

            # ---- h = sig(o) * tanh(c); decision MLP; d feedback
            th = {ph: tp.tile([128, W], BF16, tag=f"th{ph}", name=f"th{ph}")
                  for ph in PH}
            R = {ph: tp.tile([128, W], BF16, tag=f"R{ph}", name=f"R{ph}")
                 for ph in PH}
            Y = {ph: py.tile([4, W], F32, tag=f"Y{ph}", name=f"Y{ph}")
                 for ph in PH}
            for ph in PH:
                nc.scalar.activation(th[ph][:], cst[:, col[ph]], AF.Tanh)
            for ph in PH:
                nc.gpsimd.tensor_mul(hst[:, col[ph]], S[ph][:, 3 * W : 4 * W],
                                     th[ph][:])
            for ph in PH:
                nc.tensor.matmul(M1[ph][:], wm1[:], hst[:, col[ph]],
                                 start=True, stop=True, skip_group_check=True)
            for ph in PH:
                nc.vector.tensor_scalar(R[ph][:], M1[ph][:], b1c[:, 0:1],
                                        0.0, ALU.add, ALU.max)
            for ph in PH:
                nc.tensor.matmul(Y[ph][:], wm2[:], R[ph][:],
                                 start=True, stop=True, skip_group_check=True)
            for ph in PH:
                # d = tanh(0.5 y + 0.5 b2) into next step's z column
                nc.scalar.activation(drows(zbn)[:, ncolr[ph]], Y[ph][:],
                                     AF.Tanh, bias=b2h[0:4, 0:1], scale=0.5)

            # ---- dump this block's d-rows once its last column is written
            if s == SB - 1:
                nc.sync.dma_start(d_out[blk], drows(zbt))

        # Step 0 is emitted phase-sequentially with phase B's d-init
        # depending on phase A's first activations: the two otherwise
        # independent chains start ~half a step apart and stay offset.
        emit_step(0, [0])
        nc.scalar.activation(stagger[0], wxd[0][32:36, 0:N - N // NPH],
                             AF.Copy, bias=-1.0, scale=0.0)
        emit_step(0, [1])
        for t in range(1, T):
            emit_step(t, list(range(NPH)))

        # final d (t = T-1) lives in dtail column 0
        nc.sync.dma_start(d_out[T // SB, :, 0:N], drows(dtail)[:, 0:N])

    return d_out


def gather_out(scratch, cfg):
    """d_scratch [T/SB+1, 4, SB*N] (tanh form, shifted by one) -> [B, T] f32."""
    T, SB, N = cfg.T, cfg.sblk, cfg.ncol
    sc = np.asarray(scratch).astype(np.float32).reshape(T // SB + 1, 4, SB, N)
    seq = sc.transpose(1, 3, 0, 2).reshape(4, N, (T // SB + 1) * SB)
    vals = seq[:, :, 1 : T + 1]                   # drop the t=-1 init slot
    return (0.5 + 0.5 * vals).reshape(cfg.B, T).astype(np.float32)


def make_in_map(x_core, folded, b2h, cfg):
    m = {"xprep": prep_x(x_core, cfg), "wm1": folded["wm1"],
         "wm2": folded["wm2"], "b1col": folded["b1col"], "b2half": b2h}
    for g in range(4):
        m[f"wx{g}"] = folded["wx"][g]
        m[f"wxd{g}"] = folded["wxd"][g]
        m[f"whh{g}"] = folded["whh"][g]
    return m


# ----------------------------------------------------------------------------
# numpy reference of the exact model math (for mini-tests)
# ----------------------------------------------------------------------------
def numpy_model(x, params):
    """x [B, T, 4] -> [B, T] float32, same math as reference()."""
    B, T, F = x.shape
    H = params["W_hh"].shape[1]
    inv_std = 1.0 / np.sqrt(params["bn_var"] + EPS)

    h = np.zeros((B, H), np.float32)
    c = np.zeros((B, H), np.float32)
    d = np.zeros((B, 1), np.float32)
    outs = np.zeros((B, T), np.float32)
    sig = lambda v: 1.0 / (1.0 + np.exp(-v))
    for t in range(T):
        z = np.concatenate([x[:, t], d], 1)
        z = (z - params["bn_mean"]) * inv_std * params["bn_gamma"] + params["bn_beta"]
        gates = z @ params["W_ih"].T + params["b_ih"] + h @ params["W_hh"].T + params["b_hh"]
        i, f, g, o = np.split(gates, 4, 1)
        c = sig(f) * c + sig(i) * np.tanh(g)
        h = sig(o) * np.tanh(c)
        d = sig(np.maximum(h @ params["W1"].T + params["b1"], 0) @ params["W2"].T + params["b2"])
        outs[:, t] = d[:, 0]
    return outs


# ----------------------------------------------------------------------------
# Entry point
# ----------------------------------------------------------------------------
_CACHE = {}


def _get_compiled():
    if "nc" not in _CACHE:
        nc = bacc.Bacc("TRN2", target_bir_lowering=False, debug=False)
        build_kernel(nc, FULL)
        nc.compile()
        _CACHE["nc"] = nc
    return _CACHE["nc"]


def kernel(**inputs):
    from concourse.bass_utils import run_bass_kernel_spmd

    x = np.asarray(inputs["x"], np.float32)
    B, T, F = x.shape
    ncores = 8
    bc = B // ncores
    folded = fold_params(inputs)
    b2h = np.full((128, 1), folded["b2half"], np.float32)

    nc = _get_compiled()
    in_maps = [make_in_map(x[c * bc : (c + 1) * bc], folded, b2h, FULL)
               for c in range(ncores)]

    res = run_bass_kernel_spmd(nc, in_maps, list(range(ncores)))
    outs = [gather_out(res.results[c]["d_scratch"], FULL) for c in range(ncores)]
    return np.concatenate(outs, 0)[:, :, None].astype(np.float32)

